# revision 1
# baseline (speedup 1.0000x reference)
"""Trainium2 Bass kernel for a 2-layer Mamba stack (BasicLayer).

Per layer: LayerNorm -> in_proj (1024->4096) -> causal depthwise conv(k=4)
+ SiLU -> x_proj (2048->96) -> dt_proj + softplus -> selective scan over
L=2048 -> gate with SiLU(z) -> out_proj (2048->1024).

Sharding: tensor-parallel over d_inner (2048 / 8 cores = 256 channels per
core).  The selective scan is independent per channel, so each core scans
its own channels.  Cross-core sums (x_proj contraction and out_proj
contraction over d_inner) are AllReduced on-chip, split per batch so the
collectives overlap with compute.  Weights are pre-sliced and
pre-transposed on the host (pure data movement); all math runs on device.

The scan recurrence h_t = exp(dt*A)*h_{t-1} + (dt*u*B)_t runs on the DVE
tensor_tensor_scan instruction (fp32 internal state).  B_t / C_t rows are
replicated across partitions with broadcast DMAs.  softplus is computed
log-free via exp + series + Newton iterations (no Softplus/Ln table on
this hardware).
"""

import numpy as np

try:
    import concourse.bass as bass
except ImportError:  # pragma: no cover - fallback for odd sys.path setups
    import sys

    sys.path.insert(0, "/opt/trn_rl_repo")
    import concourse.bass as bass

import concourse.bacc as bacc
import concourse.mybir as mybir
import concourse.tile as tile
from concourse.bass_utils import run_bass_kernel_spmd

F32 = mybir.dt.float32
BF16 = mybir.dt.bfloat16
AF = mybir.ActivationFunctionType
ALU = mybir.AluOpType

# Problem shapes (hardcoded per the contract)
B, L = 2, 2048
DM, DI, DS, DTR, DCONV, DEPTH = 1024, 2048, 16, 64, 4, 2
EPS = 1e-5
NCORES = 8
DL = DI // NCORES          # 256 channels per core
NDT = DL // 128            # 2 channel tiles per core
T = B * L                  # 4096 tokens
NCH = T // 512             # 8 chunks of 512 tokens


def build_nc(apply_norm_w: bool, apply_norm_b: bool, fake_cc: bool = False,
             scan_bf16: bool = True):
    nc = bacc.Bacc(
        "TRN2",
        target_bir_lowering=False,
        debug=False,
        enable_asserts=False,
        num_devices=NCORES,
    )

    sdt = BF16 if scan_bf16 else F32

    # ---- I/O declarations (per-core data supplied via in_maps) ----
    x_dram = nc.dram_tensor("x_tm", [T, DM], F32, kind="ExternalInput")
    w_inT = nc.dram_tensor("w_inT", [DEPTH, DM, 4 * 128], F32, kind="ExternalInput")
    w_outT = nc.dram_tensor("w_outT", [DEPTH, DL, DM], F32, kind="ExternalInput")
    w_xpT = nc.dram_tensor("w_xpT", [DEPTH, DL, 96], F32, kind="ExternalInput")
    w_dtT = nc.dram_tensor("w_dtT", [DEPTH, DTR, DL], F32, kind="ExternalInput")
    conv_w = nc.dram_tensor("conv_w_c", [DEPTH, DL, DCONV], F32, kind="ExternalInput")
    conv_b = nc.dram_tensor("conv_b_c", [DEPTH, DL, 1], F32, kind="ExternalInput")
    dt_b = nc.dram_tensor("dt_b_c", [DEPTH, DL, 1], F32, kind="ExternalInput")
    a_log = nc.dram_tensor("a_log_c", [DEPTH, DL, DS], F32, kind="ExternalInput")
    d_p = nc.dram_tensor("d_c", [DEPTH, DL, 1], F32, kind="ExternalInput")
    ident = nc.dram_tensor("ident", [128, 128], F32, kind="ExternalInput")
    ones1 = nc.dram_tensor("ones1", [1, 128], F32, kind="ExternalInput")
    if apply_norm_w:
        nwb = nc.dram_tensor("norm_w_bc", [DEPTH, 128, DM], F32, kind="ExternalInput")
    if apply_norm_b:
        nbb = nc.dram_tensor("norm_b_bc", [DEPTH, 128, DM], F32, kind="ExternalInput")
    out_dram = nc.dram_tensor("out_tm", [T, DM], F32, kind="ExternalOutput")

    groups = [list(range(NCORES))]

    def all_reduce(src_ap, dst_ap):
        if fake_cc:
            nc.sync.dma_start(dst_ap, src_ap)
        else:
            nc.gpsimd.collective_compute(
                "AllReduce", ALU.add, replica_groups=groups,
                ins=[src_ap], outs=[dst_ap],
            )

    with tile.TileContext(nc, num_cores=NCORES) as tc:
        with (
            tc.tile_pool(name="wp", bufs=1) as wp,
            tc.tile_pool(name="lnp", bufs=1) as lnp,
            tc.tile_pool(name="sp", bufs=1) as sp,
            tc.tile_pool(name="dp", bufs=1) as dp,
            tc.tile_pool(name="dram", bufs=2, space="DRAM") as dram,
        ):
            ident_sb = wp.tile([128, 128], F32, tag="ident")
            nc.sync.dma_start(ident_sb[:], ident[:, :])
            idacc = ident_sb
            if scan_bf16:
                idbf = wp.tile([128, 128], BF16, tag="idbf")
                nc.vector.tensor_copy(idbf[:], ident_sb[:])
                idacc = idbf
            eps_sb = wp.tile([128, 1], F32, tag="eps")
            nc.vector.memset(eps_sb[:], EPS)

            hsrc = [x_dram.ap()[0:L, :], x_dram.ap()[L:T, :]]
            psA_cm = tc.tile_pool(name="psA", bufs=2, space="PSUM")
            psA = psA_cm.__enter__()
            psD_cm = tc.tile_pool(name="psD", bufs=1, space="PSUM")
            psD = psD_cm.__enter__()
            for l in range(DEPTH):
                # ---- per-layer weights ----
                winT = []
                for kt in range(8):
                    w = wp.tile([128, 512], F32, tag=f"winT{kt}")
                    nc.sync.dma_start(w[:], w_inT[l, kt * 128:(kt + 1) * 128, :])
                    winT.append(w)
                woutT = []
                for j in range(NDT):
                    w = wp.tile([128, DM], F32, tag=f"woutT{j}")
                    nc.sync.dma_start(w[:], w_outT[l, j * 128:(j + 1) * 128, :])
                    woutT.append(w)
                wxpT = []
                for j in range(NDT):
                    w = wp.tile([128, 96], F32, tag=f"wxpT{j}")
                    nc.sync.dma_start(w[:], w_xpT[l, j * 128:(j + 1) * 128, :])
                    wxpT.append(w)
                wdtT = wp.tile([DTR, DL], F32, tag="wdtT")
                nc.sync.dma_start(wdtT[:], w_dtT[l, :, :])
                convw, convb, dtb, Dp, Asb = [], [], [], [], []
                for j in range(NDT):
                    cw = wp.tile([128, DCONV], F32, tag=f"convw{j}")
                    nc.sync.dma_start(cw[:], conv_w[l, j * 128:(j + 1) * 128, :])
                    convw.append(cw)
                    cb = wp.tile([128, 1], F32, tag=f"convb{j}")
                    nc.sync.dma_start(cb[:], conv_b[l, j * 128:(j + 1) * 128, :])
                    convb.append(cb)
                    db = wp.tile([128, 1], F32, tag=f"dtb{j}")
                    nc.sync.dma_start(db[:], dt_b[l, j * 128:(j + 1) * 128, :])
                    dtb.append(db)
                    dd = wp.tile([128, 1], F32, tag=f"dd{j}")
                    nc.sync.dma_start(dd[:], d_p[l, j * 128:(j + 1) * 128, :])
                    Dp.append(dd)
                    at = wp.tile([128, DS], F32, tag=f"alog{j}")
                    nc.sync.dma_start(at[:], a_log[l, j * 128:(j + 1) * 128, :])
                    ae = wp.tile([128, DS], F32, tag=f"aexp{j}")
                    nc.scalar.activation(ae[:], at[:], AF.Exp)
                    an = wp.tile([128, DS], F32, tag=f"aneg{j}")
                    nc.vector.tensor_scalar_mul(an[:], ae[:], -1.0)
                    Asb.append(an)
                if apply_norm_w:
                    nw_sb = wp.tile([128, DM], F32, tag="nwsb")
                    nc.sync.dma_start(nw_sb[:], nwb[l, :, :])
                if apply_norm_b:
                    nb_sb = wp.tile([128, DM], F32, tag="nbsb")
                    nc.sync.dma_start(nb_sb[:], nbb[l, :, :])

                # ---- DRAM staging for this layer ----
                u_st = dram.tile([DL, T], F32, tag="ust")
                y_st = dram.tile([DL, T], F32, tag="yst")
                z_st = dram.tile([DL, T], F32, tag="zst")
                xdbl_in = [dram.tile([96, L], F32, tag=f"xdbli{b}", name=f"xdbli{l}_{b}") for b in range(B)]
                xdbl_sh = [dram.tile([96, L], F32, tag=f"xdblo{b}", addr_space="Shared",
                                      name=f"xdblo{l}_{b}") for b in range(B)]
                bc_bf = [dram.tile([2 * DS, L], sdt, tag=f"bcbf{b}", name=f"bcbf{l}_{b}") for b in range(B)]
                out_part = [dram.tile([L, DM], F32, tag=f"opart{b}", name=f"opart{l}_{b}") for b in range(B)]
                hred = [dram.tile([L, DM], F32, tag=f"hred{b}", addr_space="Shared",
                                   name=f"hred{l}_{b}") for b in range(B)]

                # ================= phase A: LN + transpose + in_proj + conv =================
                x_dbl = sp.tile([96, T], F32, tag="xdbl")
                prev_uext = [None, None]
                if True:
                    for ci in range(NCH):
                        b = ci // 4
                        tok0 = ci * 512
                        hn_pack = lnp.tile([128, 4096], F32, tag="hnpack")
                        for tti in range(4):
                            row0 = (ci % 4) * 512 + tti * 128
                            xa = lnp.tile([128, DM], F32, tag="xa", bufs=2)
                            nc.sync.dma_start(xa[:], hsrc[b][row0:row0 + 128, :])
                            hcol = hn_pack[:, tti * DM:(tti + 1) * DM]
                            sums = lnp.tile([128, 1], F32, tag="sums", bufs=2)
                            nc.scalar.activation(hcol, xa[:], AF.Identity, accum_out=sums[:])
                            sumsq = lnp.tile([128, 1], F32, tag="sumsq", bufs=2)
                            nc.scalar.activation(hcol, xa[:], AF.Square, accum_out=sumsq[:])
                            mean = lnp.tile([128, 1], F32, tag="mean", bufs=2)
                            nc.vector.tensor_scalar_mul(mean[:], sums[:], 1.0 / DM)
                            msq = lnp.tile([128, 1], F32, tag="msq", bufs=2)
                            nc.vector.tensor_scalar_mul(msq[:], sumsq[:], 1.0 / DM)
                            nvar = lnp.tile([128, 1], F32, tag="nvar", bufs=2)
                            nc.vector.scalar_tensor_tensor(
                                nvar[:], mean[:], mean[:], msq[:], ALU.mult, ALU.subtract
                            )
                            std = lnp.tile([128, 1], F32, tag="std", bufs=2)
                            nc.scalar.activation(std[:], nvar[:], AF.Sqrt, bias=eps_sb[:], scale=-1.0)
                            rstd = lnp.tile([128, 1], F32, tag="rstd", bufs=2)
                            nc.vector.reciprocal(rstd[:], std[:])
                            nbias = lnp.tile([128, 1], F32, tag="nbias", bufs=2)
                            nc.vector.scalar_tensor_tensor(
                                nbias[:], mean[:], -1.0, rstd[:], ALU.mult, ALU.mult
                            )
                            if apply_norm_w or apply_norm_b:
                                hn0 = lnp.tile([128, DM], F32, tag="hn0", bufs=2)
                                nc.scalar.activation(
                                    hn0[:], xa[:], AF.Identity, bias=nbias[:], scale=rstd[:]
                                )
                                if apply_norm_w and apply_norm_b:
                                    hn1 = lnp.tile([128, DM], F32, tag="hn1", bufs=2)
                                    nc.vector.tensor_mul(hn1[:], hn0[:], nw_sb[:])
                                    nc.vector.tensor_add(hcol, hn1[:], nb_sb[:])
                                elif apply_norm_w:
                                    nc.vector.tensor_mul(hcol, hn0[:], nw_sb[:])
                                else:
                                    nc.vector.tensor_add(hcol, hn0[:], nb_sb[:])
                            else:
                                nc.scalar.activation(
                                    hcol, xa[:], AF.Identity, bias=nbias[:], scale=rstd[:]
                                )
                        hnT = []
                        for kt in range(8):
                            pt = psA.tile([128, 512], F32, tag="pt", bufs=1)
                            for tti in range(4):
                                nc.tensor.transpose(
                                    pt[:, tti * 128:(tti + 1) * 128],
                                    hn_pack[:, tti * DM + kt * 128: tti * DM + (kt + 1) * 128],
                                    ident_sb[:],
                                )
                            ht = lnp.tile([128, 512], F32, tag=f"hnT{kt}")
                            nc.any.tensor_copy(ht[:], pt[:])
                            hnT.append(ht)
                        for mt in range(4):
                            pm = psA.tile([128, 512], F32, tag="pm")
                            for kt in range(8):
                                nc.tensor.matmul(
                                    pm[:],
                                    winT[kt][:, mt * 128:(mt + 1) * 128],
                                    hnT[kt][:],
                                    start=(kt == 0),
                                    stop=(kt == 7),
                                )
                            if mt < NDT:
                                ue = sp.tile([128, 515], F32, tag=f"uext{mt}", bufs=2)
                                if ci % 4 == 0:
                                    nc.vector.memset(ue[:, 0:3], 0.0)
                                else:
                                    nc.vector.tensor_copy(
                                        ue[:, 0:3], prev_uext[mt][:, 512:515]
                                    )
                                nc.any.tensor_copy(ue[:, 3:515], pm[:])
                                prev_uext[mt] = ue
                            else:
                                zc = sp.tile([128, 512], F32, tag="zc")
                                nc.scalar.activation(zc[:], pm[:], AF.Silu)
                                nc.sync.dma_start(
                                    z_st[(mt - NDT) * 128:(mt - NDT + 1) * 128, tok0:tok0 + 512],
                                    zc[:],
                                )
                        px = psA.tile([96, 512], F32, tag="pm")
                        for j in range(NDT):
                            ue = prev_uext[j]
                            c0 = sp.tile([128, 512], F32, tag="cv0")
                            nc.vector.tensor_scalar(
                                c0[:], ue[:, 0:512], convw[j][:, 0:1], None, ALU.mult
                            )
                            c1 = sp.tile([128, 512], F32, tag="cv1")
                            nc.vector.scalar_tensor_tensor(
                                c1[:], ue[:, 1:513], convw[j][:, 1:2], c0[:], ALU.mult, ALU.add
                            )
                            c2 = sp.tile([128, 512], F32, tag="cv0")
                            nc.vector.scalar_tensor_tensor(
                                c2[:], ue[:, 2:514], convw[j][:, 2:3], c1[:], ALU.mult, ALU.add
                            )
                            c3 = sp.tile([128, 512], F32, tag="cv1")
                            nc.vector.scalar_tensor_tensor(
                                c3[:], ue[:, 3:515], convw[j][:, 3:4], c2[:], ALU.mult, ALU.add
                            )
                            uc = sp.tile([128, 512], F32, tag="uc", bufs=2)
                            nc.scalar.activation(uc[:], c3[:], AF.Silu, bias=convb[j][:])
                            nc.sync.dma_start(
                                u_st[j * 128:(j + 1) * 128, tok0:tok0 + 512], uc[:]
                            )
                            nc.tensor.matmul(
                                px[:], wxpT[j][:], uc[:], start=(j == 0), stop=(j == NDT - 1)
                            )
                        nc.any.tensor_copy(x_dbl[:, tok0:tok0 + 512], px[:])

                        # per-batch x_dbl AllReduce as soon as a batch's chunks finish
                        if ci % 4 == 3:
                            nc.sync.dma_start(xdbl_in[b][:, :], x_dbl[:, b * L:(b + 1) * L])
                            all_reduce(xdbl_in[b].opt(), xdbl_sh[b].opt())
                            # stage B/C rows (cast for the scan) back to DRAM for
                            # partition-broadcast loads
                            bcs = sp.tile([2 * DS, L], F32, tag="bcs")
                            nc.sync.dma_start(bcs[:], xdbl_sh[b][DTR:96, :])
                            bcsb = sp.tile([2 * DS, L], sdt, tag="bcsb")
                            nc.any.tensor_copy(bcsb[:], bcs[:])
                            nc.sync.dma_start(bc_bf[b][:, :], bcsb[:])

                    # ============= phases D/E: dt, scan, gate, out_proj =============
                    for b in range(B):
                        xrd = dp.tile([DTR, L], F32, tag="xrd")
                        nc.sync.dma_start(xrd[:], xdbl_sh[b][0:DTR, :])
                        for j in range(NDT):
                            dt_j = dp.tile([128, L], F32, tag="dtt")
                            # softplus(x) = log(1+e^x), log-free: y=e^x, series
                            # init, 3 Newton steps (w <- w + (1+y)e^-w - 1)
                            for hf in range(2):
                                h0 = hf * 1024
                                yv = dp.tile([128, 1024], F32, tag="sp0")
                                for q in range(2):
                                    pdm = psD.tile([128, 512], F32, tag="yps")
                                    nc.tensor.matmul(
                                        pdm[:],
                                        wdtT[:, j * 128:(j + 1) * 128],
                                        xrd[:, h0 + q * 512: h0 + (q + 1) * 512],
                                        start=True,
                                        stop=True,
                                    )
                                    nc.scalar.activation(
                                        yv[:, q * 512:(q + 1) * 512], pdm[:],
                                        AF.Exp, bias=dtb[j][:],
                                    )
                                y2s = dp.tile([128, 1024], F32, tag="sp1")
                                nc.scalar.activation(y2s[:], yv[:], AF.Square)
                                a1 = dp.tile([128, 1024], F32, tag="sp2")
                                nc.vector.tensor_scalar(a1[:], yv[:], -0.5, 1.0, ALU.mult, ALU.add)
                                a2 = dp.tile([128, 1024], F32, tag="sp3")
                                nc.vector.tensor_mul(a2[:], yv[:], a1[:])
                                a3 = dp.tile([128, 1024], F32, tag="sp2")
                                nc.vector.tensor_scalar(a3[:], yv[:], -0.25, 1.0 / 3.0, ALU.mult, ALU.add)
                                a4 = dp.tile([128, 1024], F32, tag="ada")
                                nc.vector.tensor_mul(a4[:], y2s[:], a3[:])
                                a5 = dp.tile([128, 1024], F32, tag="sp1")
                                nc.vector.tensor_mul(a5[:], yv[:], a4[:])
                                w0 = dp.tile([128, 1024], F32, tag="sp2")
                                nc.vector.tensor_add(w0[:], a2[:], a5[:])
                                w = dp.tile([128, 1024], F32, tag="sp3")
                                nc.vector.tensor_scalar_max(w[:], w0[:], 0.0)
                                for it, wtag in enumerate(["bt", None]):
                                    ew = dp.tile([128, 1024], F32, tag="ada")
                                    nc.scalar.activation(ew[:], w[:], AF.Exp, scale=-1.0)
                                    ye = dp.tile([128, 1024], F32, tag="sp1")
                                    nc.vector.tensor_mul(ye[:], yv[:], ew[:])
                                    tcv = dp.tile([128, 1024], F32, tag="sp2")
                                    nc.vector.scalar_tensor_tensor(
                                        tcv[:], ew[:], -1.0, ye[:], ALU.add, ALU.add
                                    )
                                    if wtag is None:
                                        nc.vector.tensor_add(
                                            dt_j[:, h0:h0 + 1024], w[:], tcv[:]
                                        )
                                    else:
                                        wn = dp.tile([128, 1024], F32, tag=wtag)
                                        nc.vector.tensor_add(wn[:], w[:], tcv[:])
                                        w = wn
                            ub = dp.tile([128, L], F32, tag="ub")
                            nc.sync.dma_start(
                                ub[:], u_st[j * 128:(j + 1) * 128, b * L:(b + 1) * L]
                            )
                            du = dp.tile([128, L], sdt, tag="dtu")
                            nc.vector.tensor_mul(du[:], dt_j[:], ub[:])
                            y_ps = psD.tile([128, L], F32, tag="yps")
                            for n in range(DS):
                                pb = dp.tile([128, L], sdt, tag="pbbf", bufs=2)
                                nc.sync.dma_start(
                                    pb[:], bc_bf[b][n:n + 1, :].to_broadcast((128, L))
                                )
                                pc = dp.tile([128, L], sdt, tag="pcbf", bufs=2)
                                nc.sync.dma_start(
                                    pc[:], bc_bf[b][DS + n:DS + n + 1, :].to_broadcast((128, L))
                                )
                                ada = dp.tile([128, L], sdt, tag="adas")
                                nc.scalar.activation(
                                    ada[:], dt_j[:], AF.Exp, scale=Asb[j][:, n:n + 1]
                                )
                                bt = dp.tile([128, L], sdt, tag="bt")
                                nc.vector.tensor_mul(bt[:], du[:], pb[:])
                                hs = dp.tile([128, L], sdt, tag="hs")
                                nc.vector.tensor_tensor_scan(
                                    hs[:], ada[:], bt[:], 0.0, ALU.mult, ALU.add
                                )
                                yt = dp.tile([128, L], sdt, tag="yt")
                                nc.vector.tensor_mul(yt[:], hs[:], pc[:])
                                for q in range(4):
                                    nc.tensor.matmul(
                                        y_ps[:, q * 512:(q + 1) * 512],
                                        idacc[:],
                                        yt[:, q * 512:(q + 1) * 512],
                                        start=(n == 0),
                                        stop=(n == DS - 1),
                                    )
                            ub2 = dp.tile([128, L], F32, tag="ub")
                            nc.sync.dma_start(
                                ub2[:], u_st[j * 128:(j + 1) * 128, b * L:(b + 1) * L]
                            )
                            zb = dp.tile([128, L], F32, tag="zb")
                            nc.sync.dma_start(
                                zb[:], z_st[j * 128:(j + 1) * 128, b * L:(b + 1) * L]
                            )
                            for hf in range(2):
                                h0 = hf * 1024
                                y1h = dp.tile([128, 1024], F32, tag="sp2")
                                nc.vector.scalar_tensor_tensor(
                                    y1h[:], ub2[:, h0:h0 + 1024], Dp[j][:],
                                    y_ps[:, h0:h0 + 1024], ALU.mult, ALU.add
                                )
                                y2h = dp.tile([128, 1024], F32, tag="sp3")
                                nc.vector.tensor_mul(y2h[:], y1h[:], zb[:, h0:h0 + 1024])
                                nc.sync.dma_start(
                                    y_st[j * 128:(j + 1) * 128,
                                         b * L + h0: b * L + h0 + 1024],
                                    y2h[:],
                                )
                        # out_proj for this batch
                        for tt in range(16):
                            yl = []
                            for j in range(NDT):
                                ylj = dp.tile([128, 128], F32, tag=f"yl{j}", bufs=2)
                                nc.sync.dma_start(
                                    ylj[:],
                                    y_st[j * 128:(j + 1) * 128,
                                         b * L + tt * 128: b * L + (tt + 1) * 128],
                                )
                                yl.append(ylj)
                            for nt2 in range(2):
                                po = psD.tile([128, 512], F32, tag="po")
                                for j in range(NDT):
                                    nc.tensor.matmul(
                                        po[:],
                                        yl[j][:],
                                        woutT[j][:, nt2 * 512:(nt2 + 1) * 512],
                                        start=(j == 0),
                                        stop=(j == NDT - 1),
                                    )
                                oc = dp.tile([128, 512], F32, tag="oc")
                                nc.any.tensor_copy(oc[:], po[:])
                                nc.sync.dma_start(
                                    out_part[b][tt * 128:(tt + 1) * 128,
                                                nt2 * 512:(nt2 + 1) * 512],
                                    oc[:],
                                )
                        all_reduce(out_part[b].opt(), hred[b].opt())

                hsrc = [hred[0], hred[1]]

            for b in range(B):
                nc.sync.dma_start(out_dram[b * L:(b + 1) * L, :], hsrc[b])
            psD_cm.__exit__(None, None, None)
            psA_cm.__exit__(None, None, None)

    nc.compile()
    return nc


_CACHE = {}


def _get_nc(apply_norm_w, apply_norm_b, fake_cc=False, scan_bf16=True):
    key = (apply_norm_w, apply_norm_b, fake_cc, scan_bf16)
    if key not in _CACHE:
        _CACHE[key] = build_nc(apply_norm_w, apply_norm_b, fake_cc, scan_bf16)
    return _CACHE[key]


def make_in_maps(x, norm_w, norm_b, in_proj_w, conv_w, conv_b, x_proj_w,
                 dt_proj_w, dt_proj_b, A_log, D, out_proj_w,
                 apply_norm_w, apply_norm_b):
    f = lambda a: np.ascontiguousarray(np.asarray(a), dtype=np.float32)
    x_tm = f(x).reshape(T, DM)
    in_maps = []
    for c in range(NCORES):
        sl = slice(c * DL, (c + 1) * DL)
        w_in_rows = np.concatenate(
            [np.asarray(in_proj_w)[:, sl, :], np.asarray(in_proj_w)[:, DI + c * DL: DI + (c + 1) * DL, :]],
            axis=1,
        )  # (2, 512, 1024)
        m = {
            "x_tm": x_tm,
            "w_inT": f(w_in_rows.transpose(0, 2, 1)),
            "w_outT": f(np.asarray(out_proj_w)[:, :, sl].transpose(0, 2, 1)),
            "w_xpT": f(np.asarray(x_proj_w)[:, :, sl].transpose(0, 2, 1)),
            "w_dtT": f(np.asarray(dt_proj_w)[:, sl, :].transpose(0, 2, 1)),
            "conv_w_c": f(np.asarray(conv_w)[:, sl, 0, :]),
            "conv_b_c": f(np.asarray(conv_b)[:, sl][..., None]),
            "dt_b_c": f(np.asarray(dt_proj_b)[:, sl][..., None]),
            "a_log_c": f(np.asarray(A_log)[:, sl, :]),
            "d_c": f(np.asarray(D)[:, sl][..., None]),
            "ident": np.eye(128, dtype=np.float32),
            "ones1": np.ones((1, 128), dtype=np.float32),
        }
        if apply_norm_w:
            m["norm_w_bc"] = f(np.broadcast_to(np.asarray(norm_w)[:, None, :], (DEPTH, 128, DM)))
        if apply_norm_b:
            m["norm_b_bc"] = f(np.broadcast_to(np.asarray(norm_b)[:, None, :], (DEPTH, 128, DM)))
        in_maps.append(m)
    return in_maps


def kernel(x, x_size, norm_w, norm_b, in_proj_w, conv_w, conv_b, x_proj_w,
           dt_proj_w, dt_proj_b, A_log, D, out_proj_w, **_unused):
    apply_norm_w = not np.allclose(np.asarray(norm_w), 1.0)
    apply_norm_b = not np.allclose(np.asarray(norm_b), 0.0)
    nc = _get_nc(apply_norm_w, apply_norm_b)
    in_maps = make_in_maps(
        x, norm_w, norm_b, in_proj_w, conv_w, conv_b, x_proj_w,
        dt_proj_w, dt_proj_b, A_log, D, out_proj_w,
        apply_norm_w, apply_norm_b,
    )
    res = run_bass_kernel_spmd(nc, in_maps, core_ids=list(range(NCORES)))
    return res.results[0]["out_tm"].reshape(B, L, DM)



# revision 9
# speedup vs baseline: 1.3765x; 1.3765x over previous
"""Trainium2 Bass kernel for a 2-layer Mamba stack (BasicLayer). v2.

Per layer: LayerNorm -> in_proj (1024->4096) -> causal depthwise conv(k=4)
+ SiLU -> x_proj (2048->96) -> dt_proj + softplus -> selective scan over
L=2048 -> gate with SiLU(z) -> out_proj (2048->1024).

Sharding: tensor-parallel over d_inner (2048 / 8 cores = 256 channels per
core).  Cross-core sums (x_proj and out_proj contractions) are AllReduced
on-chip in bf16, out_proj split in token halves so the collectives overlap
compute.  All matmuls run in bf16 (fp32 matmul is 4 cycles/row vs 1 for
bf16).  Transposes use the DMA xbar (dma_start_transpose) instead of the
PE+PSUM path.  The depthwise conv and the D*u skip term are expressed as
diagonal-matrix matmuls on the PE so the vector engine only carries the
scan itself plus the B/C elementwise products.  softplus(x) is computed as
Ln(1+Exp(x)) -- both functions live in the same activation table, and the
LayerNorm rstd is the only per-chunk table swap (Sqrt).
"""

import numpy as np

try:
    import concourse.bass as bass
except ImportError:  # pragma: no cover
    import sys

    sys.path.insert(0, "/opt/trn_rl_repo")
    import concourse.bass as bass

import concourse.bacc as bacc
import concourse.mybir as mybir
import concourse.tile as tile
from concourse.bass_utils import run_bass_kernel_spmd

F32 = mybir.dt.float32
BF16 = mybir.dt.bfloat16
AF = mybir.ActivationFunctionType
ALU = mybir.AluOpType

B, L = 2, 2048
DM, DI, DS, DTR, DCONV, DEPTH = 1024, 2048, 16, 64, 4, 2
EPS = 1e-5
NCORES = 8
DL = DI // NCORES          # 256 channels per core
NDT = DL // 128            # 2 channel tiles per core
T = B * L                  # 4096 tokens
NCH = T // 512             # 8 chunks of 512 tokens
LH = L // 2                # token half for out AllReduce chunking

# knobs
YT_POOL_N = set()          # scan ns whose yt-mul runs on gpsimd instead of DVE
BT_POOL_N = set()


def build_nc(apply_norm_w: bool, apply_norm_b: bool, fake_cc: bool = False):
    nc = bacc.Bacc(
        "TRN2",
        target_bir_lowering=False,
        debug=False,
        enable_asserts=False,
        num_devices=NCORES,
    )

    # ---- I/O declarations (per-core data supplied via in_maps) ----
    x_dram = nc.dram_tensor("x_tm", [T, DM], BF16, kind="ExternalInput")
    w_inT = nc.dram_tensor("w_inT", [DEPTH, DM, 4 * 128], BF16, kind="ExternalInput")
    w_outT = nc.dram_tensor("w_outT", [DEPTH, DL, DM], BF16, kind="ExternalInput")
    w_xpT = nc.dram_tensor("w_xpT", [DEPTH, DL, 96], BF16, kind="ExternalInput")
    w_dtT = nc.dram_tensor("w_dtT", [DEPTH, DTR, DL], BF16, kind="ExternalInput")
    conv_dg = nc.dram_tensor("conv_dg", [DEPTH, NDT, DCONV, 128, 128], BF16,
                             kind="ExternalInput")
    d_dg = nc.dram_tensor("d_dg", [DEPTH, NDT, 128, 128], BF16, kind="ExternalInput")
    conv_b = nc.dram_tensor("conv_b_c", [DEPTH, DL, 1], F32, kind="ExternalInput")
    dt_b = nc.dram_tensor("dt_b_c", [DEPTH, DL, 1], F32, kind="ExternalInput")
    a_log = nc.dram_tensor("a_log_c", [DEPTH, DL, DS], F32, kind="ExternalInput")
    ident = nc.dram_tensor("ident_bf", [128, 128], BF16, kind="ExternalInput")
    if apply_norm_w:
        nwb = nc.dram_tensor("norm_w_bc", [DEPTH, 128, DM], F32, kind="ExternalInput")
    if apply_norm_b:
        nbb = nc.dram_tensor("norm_b_bc", [DEPTH, 128, DM], F32, kind="ExternalInput")
    out_dram = nc.dram_tensor("out_tm", [T, DM], F32, kind="ExternalOutput")

    groups = [list(range(NCORES))]

    def all_reduce(src_ap, dst_ap):
        if fake_cc:
            nc.sync.dma_start(dst_ap, src_ap)
        else:
            nc.gpsimd.collective_compute(
                "AllReduce", ALU.add, replica_groups=groups,
                ins=[src_ap], outs=[dst_ap],
            )

    with tile.TileContext(nc, num_cores=NCORES) as tc:
        with (
            tc.tile_pool(name="wp", bufs=1) as wp,
            tc.tile_pool(name="lnp", bufs=2) as lnp,
            tc.tile_pool(name="sp", bufs=2) as sp,
            tc.tile_pool(name="dp", bufs=2) as dp,
            tc.tile_pool(name="bbp", bufs=3) as bbp,
            tc.tile_pool(name="bcp", bufs=3) as bcp,
            tc.tile_pool(name="psA", bufs=2, space="PSUM") as psA,
            tc.tile_pool(name="psY", bufs=1, space="PSUM") as psY,
            tc.tile_pool(name="psS", bufs=2, space="PSUM") as psS,
            tc.tile_pool(name="dram", bufs=2, space="DRAM") as dram,
        ):
            ident_sb = wp.tile([128, 128], BF16, tag="ident")
            nc.sync.dma_start(ident_sb[:], ident[:, :])
            eps_sb = wp.tile([128, 1], F32, tag="eps")
            nc.vector.memset(eps_sb[:], EPS)
            one_sb = wp.tile([128, 1], F32, tag="one")
            nc.vector.memset(one_sb[:], 1.0)

            # hsrc(b, row0) -> AP of 128 input rows for this layer
            hsrc_l0 = [x_dram.ap()[0:L, :], x_dram.ap()[L:T, :]]

            def hsrc_l0_get(b, row0):
                return hsrc_l0[b][row0:row0 + 128, :]

            hget = hsrc_l0_get

            for l in range(DEPTH):
                # ---- per-layer weights ----
                winT = []
                for kt in range(8):
                    w = wp.tile([128, 512], BF16, tag=f"winT{kt}")
                    nc.sync.dma_start(w[:], w_inT[l, kt * 128:(kt + 1) * 128, :])
                    winT.append(w)
                woutT = []
                for j in range(NDT):
                    w = wp.tile([128, DM], BF16, tag=f"woutT{j}")
                    nc.sync.dma_start(w[:], w_outT[l, j * 128:(j + 1) * 128, :])
                    woutT.append(w)
                wxpT = []
                for j in range(NDT):
                    w = wp.tile([128, 96], BF16, tag=f"wxpT{j}")
                    nc.sync.dma_start(w[:], w_xpT[l, j * 128:(j + 1) * 128, :])
                    wxpT.append(w)
                wdtT = wp.tile([DTR, DL], BF16, tag="wdtT")
                nc.sync.dma_start(wdtT[:], w_dtT[l, :, :])
                cdg, ddg, convb, dtb, Asb = [], [], [], [], []
                for j in range(NDT):
                    row = []
                    for k in range(DCONV):
                        cw = wp.tile([128, 128], BF16, tag=f"cdg{j}_{k}")
                        nc.sync.dma_start(cw[:], conv_dg[l, j, k, :, :])
                        row.append(cw)
                    cdg.append(row)
                    dd = wp.tile([128, 128], BF16, tag=f"ddg{j}")
                    nc.sync.dma_start(dd[:], d_dg[l, j, :, :])
                    ddg.append(dd)
                    cb = wp.tile([128, 1], F32, tag=f"convb{j}")
                    nc.sync.dma_start(cb[:], conv_b[l, j * 128:(j + 1) * 128, :])
                    convb.append(cb)
                    db = wp.tile([128, 1], F32, tag=f"dtb{j}")
                    nc.sync.dma_start(db[:], dt_b[l, j * 128:(j + 1) * 128, :])
                    dtb.append(db)
                    at = wp.tile([128, DS], F32, tag=f"alog{j}")
                    nc.sync.dma_start(at[:], a_log[l, j * 128:(j + 1) * 128, :])
                    ae = wp.tile([128, DS], F32, tag=f"aexp{j}")
                    nc.scalar.activation(ae[:], at[:], AF.Exp)
                    an = wp.tile([128, DS], F32, tag=f"aneg{j}")
                    nc.vector.tensor_scalar_mul(an[:], ae[:], -1.0)
                    Asb.append(an)
                if apply_norm_w:
                    nw_sb = wp.tile([128, DM], F32, tag="nwsb")
                    nc.sync.dma_start(nw_sb[:], nwb[l, :, :])
                if apply_norm_b:
                    nb_sb = wp.tile([128, DM], F32, tag="nbsb")
                    nc.sync.dma_start(nb_sb[:], nbb[l, :, :])

                # per-layer resident SBUF tensors
                u_sb = [wp.tile([128, T], BF16, tag=f"u{j}", name=f"u{l}_{j}")
                        for j in range(NDT)]
                z_sb = [wp.tile([128, T], BF16, tag=f"z{j}", name=f"z{l}_{j}")
                        for j in range(NDT)]

                # DRAM staging for collectives
                xdbl_in = [dram.tile([96, L], BF16, tag=f"xdbli{b}",
                                     name=f"xdbli{l}_{b}") for b in range(B)]
                xdbl_sh = [dram.tile([96, L], BF16, tag=f"xdblo{b}", addr_space="Shared",
                                     name=f"xdblo{l}_{b}") for b in range(B)]
                out_part = [[dram.tile([LH, DM], BF16, tag=f"opart{b}_{h}",
                                       name=f"opart{l}_{b}_{h}") for h in range(2)]
                            for b in range(B)]
                hred = [[dram.tile([LH, DM], BF16, tag=f"hred{b}_{h}", addr_space="Shared",
                                   name=f"hred{l}_{b}_{h}") for h in range(2)]
                        for b in range(B)]

                # ================= phase A: LN + transpose + in_proj + conv ===========
                prev_ue = [None, None]
                for ci in range(NCH):
                    b = ci // 4
                    tok0 = ci * 512
                    xa_t, rstd_t, nbias_t = [], [], []
                    # pass 1: stats (Identity+Square accumulate; any act table)
                    for tti in range(4):
                        row0 = (ci % 4) * 512 + tti * 128
                        xa = lnp.tile([128, DM], BF16, tag="xa", bufs=5)
                        nc.sync.dma_start(xa[:], hget(b, row0))
                        xa_t.append(xa)
                        scr = lnp.tile([128, DM], BF16, tag="scr", bufs=2)
                        sums = lnp.tile([128, 1], F32, tag="sums", bufs=4)
                        nc.scalar.activation(scr[:], xa[:], AF.Identity, accum_out=sums[:])
                        sumsq = lnp.tile([128, 1], F32, tag="sumsq", bufs=4)
                        nc.scalar.activation(scr[:], xa[:], AF.Square, accum_out=sumsq[:])
                        mean = lnp.tile([128, 1], F32, tag="mean", bufs=4)
                        nc.vector.tensor_scalar_mul(mean[:], sums[:], 1.0 / DM)
                        msq = lnp.tile([128, 1], F32, tag="msq", bufs=4)
                        nc.vector.tensor_scalar_mul(msq[:], sumsq[:], 1.0 / DM)
                        nvar = lnp.tile([128, 1], F32, tag="nvar", bufs=4)
                        nc.vector.scalar_tensor_tensor(
                            nvar[:], mean[:], mean[:], msq[:], ALU.mult, ALU.subtract
                        )
                        rstd_t.append(nvar)  # placeholder; sqrt below (clustered)
                        nbias_t.append(mean)
                    # pass 2: rstd (Sqrt -- single table-swap cluster per chunk)
                    hn_pack = lnp.tile([128, 4096], BF16, tag="hnpack")
                    for tti in range(4):
                        nvar = rstd_t[tti]
                        mean = nbias_t[tti]
                        std = lnp.tile([128, 1], F32, tag="std", bufs=4)
                        nc.scalar.activation(std[:], nvar[:], AF.Sqrt,
                                             bias=eps_sb[:], scale=-1.0)
                        rstd = lnp.tile([128, 1], F32, tag="rstd", bufs=4)
                        nc.vector.reciprocal(rstd[:], std[:])
                        nbias = lnp.tile([128, 1], F32, tag="nbias", bufs=4)
                        nc.vector.scalar_tensor_tensor(
                            nbias[:], mean[:], -1.0, rstd[:], ALU.mult, ALU.mult
                        )
                        hcol = hn_pack[:, tti * DM:(tti + 1) * DM]
                        if apply_norm_w or apply_norm_b:
                            hn0 = lnp.tile([128, DM], F32, tag="hn0", bufs=2)
                            nc.vector.tensor_scalar(
                                hn0[:], xa_t[tti][:], rstd[:], nbias[:],
                                ALU.mult, ALU.add,
                            )
                            if apply_norm_w and apply_norm_b:
                                hn1 = lnp.tile([128, DM], F32, tag="hn1", bufs=2)
                                nc.vector.tensor_mul(hn1[:], hn0[:], nw_sb[:])
                                nc.vector.tensor_add(hcol, hn1[:], nb_sb[:])
                            elif apply_norm_w:
                                nc.vector.tensor_mul(hcol, hn0[:], nw_sb[:])
                            else:
                                nc.vector.tensor_add(hcol, hn0[:], nb_sb[:])
                        else:
                            nc.vector.tensor_scalar(
                                hcol, xa_t[tti][:], rstd[:], nbias[:],
                                ALU.mult, ALU.add,
                            )
                    # transpose via DMA xbar: hnT[p, kt, t] = hn_pack[t', kt*128+p]
                    hnT = lnp.tile([128, 8, 512], BF16, tag="hnT")
                    for tti in range(4):
                        nc.sync.dma_start_transpose(
                            hnT[:, :, tti * 128:(tti + 1) * 128],
                            hn_pack[:, tti * DM:(tti + 1) * DM],
                        )
                    # in_proj + conv + silu + x_proj
                    for mt in range(4):
                        pm = psA.tile([128, 512], F32, tag="pm")
                        for kt in range(8):
                            nc.tensor.matmul(
                                pm[:],
                                winT[kt][:, mt * 128:(mt + 1) * 128],
                                hnT[:, kt, :],
                                start=(kt == 0),
                                stop=(kt == 7),
                            )
                        if mt < NDT:
                            j = mt
                            ue = sp.tile([128, 515], BF16, tag=f"ue{j}", bufs=2)
                            if ci % 4 == 0:
                                nc.vector.memset(ue[:, 0:3], 0.0)
                            else:
                                nc.vector.tensor_copy(
                                    ue[:, 0:3], prev_ue[j][:, 512:515]
                                )
                            nc.scalar.copy(ue[:, 3:515], pm[:])
                            prev_ue[j] = ue
                            pcv = psA.tile([128, 512], F32, tag="pm")
                            for k in range(DCONV):
                                nc.tensor.matmul(
                                    pcv[:], cdg[j][k][:], ue[:, k:k + 512],
                                    start=(k == 0), stop=(k == DCONV - 1),
                                )
                            nc.scalar.activation(
                                u_sb[j][:, tok0:tok0 + 512], pcv[:],
                                AF.Silu, bias=convb[j][:],
                            )
                        else:
                            j = mt - NDT
                            nc.scalar.activation(
                                z_sb[j][:, tok0:tok0 + 512], pm[:], AF.Silu
                            )
                    px = psA.tile([96, 512], F32, tag="pm")
                    for j in range(NDT):
                        nc.tensor.matmul(
                            px[:], wxpT[j][:], u_sb[j][:, tok0:tok0 + 512],
                            start=(j == 0), stop=(j == NDT - 1),
                        )
                    xdc = sp.tile([96, 512], BF16, tag="xdc", bufs=2)
                    nc.scalar.copy(xdc[:], px[:])
                    ctok = (ci % 4) * 512
                    nc.sync.dma_start(xdbl_in[b][:, ctok:ctok + 512], xdc[:])

                    if ci % 4 == 3:
                        all_reduce(xdbl_in[b].opt(), xdbl_sh[b].opt())

                # ============= phase D: dt + scan; phase E: out_proj =============
                for b in range(B):
                    xrd = sp.tile([DTR, L], BF16, tag="xrd")
                    nc.sync.dma_start(xrd[:], xdbl_sh[b][0:DTR, :])
                    dts, dus = [], []
                    for j in range(NDT):
                        dt_j = dp.tile([128, L], BF16, tag=f"dt{j}", bufs=1)
                        for q in range(4):
                            pdm = psS.tile([128, 512], F32, tag="ps")
                            nc.tensor.matmul(
                                pdm[:],
                                wdtT[:, j * 128:(j + 1) * 128],
                                xrd[:, q * 512:(q + 1) * 512],
                                start=True, stop=True,
                            )
                            ev = sp.tile([128, 512], F32, tag="ev", bufs=2)
                            nc.scalar.activation(ev[:], pdm[:], AF.Exp, bias=dtb[j][:])
                            nc.scalar.activation(
                                dt_j[:, q * 512:(q + 1) * 512], ev[:],
                                AF.Ln, bias=one_sb[:],
                            )
                        du_j = dp.tile([128, L], BF16, tag=f"du{j}", bufs=1)
                        nc.vector.tensor_mul(
                            du_j[:], dt_j[:], u_sb[j][:, b * L:(b + 1) * L]
                        )
                        dts.append(dt_j)
                        dus.append(du_j)
                    for j in range(NDT):
                        y_ps = psY.tile([128, L], F32, tag="yps")
                        for n in range(DS):
                            pb = bbp.tile([128, L], BF16, tag="pb")
                            nc.sync.dma_start(
                                pb[:],
                                xdbl_sh[b][DTR + n:DTR + n + 1, :].to_broadcast((128, L)),
                            )
                            pc = bcp.tile([128, L], BF16, tag="pc")
                            nc.sync.dma_start(
                                pc[:],
                                xdbl_sh[b][DTR + DS + n:DTR + DS + n + 1, :]
                                .to_broadcast((128, L)),
                            )
                            ada = dp.tile([128, L], BF16, tag="ada")
                            nc.scalar.activation(
                                ada[:], dts[j][:], AF.Exp, scale=Asb[j][:, n:n + 1]
                            )
                            bt = dp.tile([128, L], BF16, tag="bt")
                            if n in BT_POOL_N:
                                nc.gpsimd.tensor_tensor(bt[:], dus[j][:], pb[:], ALU.mult)
                            else:
                                nc.vector.tensor_mul(bt[:], dus[j][:], pb[:])
                            hs = dp.tile([128, L], BF16, tag="hs")
                            nc.vector.tensor_tensor_scan(
                                hs[:], ada[:], bt[:], 0.0, ALU.mult, ALU.add
                            )
                            if n in YT_POOL_N:
                                nc.gpsimd.tensor_tensor(hs[:], hs[:], pc[:], ALU.mult)
                            else:
                                nc.vector.tensor_mul(hs[:], hs[:], pc[:])
                            for q in range(4):
                                nc.tensor.matmul(
                                    y_ps[:, q * 512:(q + 1) * 512],
                                    ident_sb[:],
                                    hs[:, q * 512:(q + 1) * 512],
                                    start=(n == 0), stop=False,
                                )
                        # D*u skip term closes the accumulation
                        for q in range(4):
                            nc.tensor.matmul(
                                y_ps[:, q * 512:(q + 1) * 512],
                                ddg[j][:],
                                u_sb[j][:, b * L + q * 512: b * L + (q + 1) * 512],
                                start=False, stop=True,
                            )
                        y2 = dp.tile([128, L], BF16, tag=f"y2{j}", bufs=1)
                        nc.vector.tensor_mul(
                            y2[:], y_ps[:], z_sb[j][:, b * L:(b + 1) * L]
                        )
                        dts[j] = None
                        if j == 0:
                            y2s = [y2]
                        else:
                            y2s.append(y2)
                    # phase E: out_proj in token halves, AllReduce each half
                    for h in range(2):
                        for tt in range(8):
                            t0 = h * LH + tt * 128
                            for nt in range(2):
                                po = psS.tile([128, 512], F32, tag="ps")
                                for j in range(NDT):
                                    nc.tensor.matmul(
                                        po[:],
                                        y2s[j][:, t0:t0 + 128],
                                        woutT[j][:, nt * 512:(nt + 1) * 512],
                                        start=(j == 0), stop=(j == NDT - 1),
                                    )
                                oc = sp.tile([128, 512], BF16, tag="oc", bufs=3)
                                nc.scalar.copy(oc[:], po[:])
                                nc.sync.dma_start(
                                    out_part[b][h][tt * 128:(tt + 1) * 128,
                                                   nt * 512:(nt + 1) * 512],
                                    oc[:],
                                )
                        all_reduce(out_part[b][h].opt(), hred[b][h].opt())

                def mk_hget(hred_l):
                    def _g(b, row0):
                        h = row0 // LH
                        r = row0 % LH
                        return hred_l[b][h][r:r + 128, :]
                    return _g

                hget = mk_hget(hred)

            # final: cast bf16 -> f32 and store
            for b in range(B):
                for h in range(2):
                    for rt in range(LH // 128):
                        ld = sp.tile([128, DM], BF16, tag="fld", bufs=2)
                        nc.sync.dma_start(ld[:], hred[b][h][rt * 128:(rt + 1) * 128, :])
                        fc = sp.tile([128, DM], F32, tag="ffc", bufs=2)
                        if rt % 2 == 0:
                            nc.scalar.copy(fc[:], ld[:])
                        else:
                            nc.vector.tensor_copy(fc[:], ld[:])
                        nc.sync.dma_start(
                            out_dram[b * L + h * LH + rt * 128:
                                     b * L + h * LH + (rt + 1) * 128, :],
                            fc[:],
                        )

    nc.compile()
    return nc


_CACHE = {}


def _get_nc(apply_norm_w, apply_norm_b, fake_cc=False):
    key = (apply_norm_w, apply_norm_b, fake_cc)
    if key not in _CACHE:
        _CACHE[key] = build_nc(apply_norm_w, apply_norm_b, fake_cc)
    return _CACHE[key]


def make_in_maps(x, norm_w, norm_b, in_proj_w, conv_w, conv_b, x_proj_w,
                 dt_proj_w, dt_proj_b, A_log, D, out_proj_w,
                 apply_norm_w, apply_norm_b):
    bf = mybir.dt.np(BF16)
    f = lambda a: np.ascontiguousarray(np.asarray(a), dtype=np.float32)
    fb = lambda a: np.ascontiguousarray(np.asarray(a, dtype=np.float32).astype(bf))
    x_tm = fb(np.asarray(x).reshape(T, DM))
    in_proj_w = np.asarray(in_proj_w)
    conv_w = np.asarray(conv_w)
    D_np = np.asarray(D)
    in_maps = []
    for c in range(NCORES):
        sl = slice(c * DL, (c + 1) * DL)
        w_in_rows = np.concatenate(
            [in_proj_w[:, sl, :], in_proj_w[:, DI + c * DL: DI + (c + 1) * DL, :]],
            axis=1,
        )  # (DEPTH, 512, 1024)
        cdg = np.zeros((DEPTH, NDT, DCONV, 128, 128), dtype=np.float32)
        ddg = np.zeros((DEPTH, NDT, 128, 128), dtype=np.float32)
        for li in range(DEPTH):
            for j in range(NDT):
                ch = slice(c * DL + j * 128, c * DL + (j + 1) * 128)
                for k in range(DCONV):
                    np.fill_diagonal(cdg[li, j, k], conv_w[li, ch, 0, k])
                np.fill_diagonal(ddg[li, j], D_np[li, ch])
        m = {
            "x_tm": x_tm,
            "w_inT": fb(w_in_rows.transpose(0, 2, 1)),
            "w_outT": fb(np.asarray(out_proj_w)[:, :, sl].transpose(0, 2, 1)),
            "w_xpT": fb(np.asarray(x_proj_w)[:, :, sl].transpose(0, 2, 1)),
            "w_dtT": fb(np.asarray(dt_proj_w)[:, sl, :].transpose(0, 2, 1)),
            "conv_dg": fb(cdg),
            "d_dg": fb(ddg),
            "conv_b_c": f(np.asarray(conv_b)[:, sl][..., None]),
            "dt_b_c": f(np.asarray(dt_proj_b)[:, sl][..., None]),
            "a_log_c": f(np.asarray(A_log)[:, sl, :]),
            "ident_bf": np.eye(128, dtype=np.float32).astype(bf),
        }
        if apply_norm_w:
            m["norm_w_bc"] = f(np.broadcast_to(np.asarray(norm_w)[:, None, :], (DEPTH, 128, DM)))
        if apply_norm_b:
            m["norm_b_bc"] = f(np.broadcast_to(np.asarray(norm_b)[:, None, :], (DEPTH, 128, DM)))
        in_maps.append(m)
    return in_maps


def kernel(x, x_size, norm_w, norm_b, in_proj_w, conv_w, conv_b, x_proj_w,
           dt_proj_w, dt_proj_b, A_log, D, out_proj_w, **_unused):
    apply_norm_w = not np.allclose(np.asarray(norm_w), 1.0)
    apply_norm_b = not np.allclose(np.asarray(norm_b), 0.0)
    nc = _get_nc(apply_norm_w, apply_norm_b)
    in_maps = make_in_maps(
        x, norm_w, norm_b, in_proj_w, conv_w, conv_b, x_proj_w,
        dt_proj_w, dt_proj_b, A_log, D, out_proj_w,
        apply_norm_w, apply_norm_b,
    )
    res = run_bass_kernel_spmd(nc, in_maps, core_ids=list(range(NCORES)))
    return res.results[0]["out_tm"].reshape(B, L, DM).astype(np.float32)
